# revision 17
# baseline (speedup 1.0000x reference)
"""GCN (2x GCNConv + MLP head) on 8 TRN2 NeuronCores via Bass/Tile.

Distribution (graph-parallel, per the node-sharding scheme):
  - nodes sharded by id across 8 cores (12500 each); weights replicated.
  - h buffers are bf16 pair-rows [npad/2, 128]: row r = nodes (2r, 2r+1),
    256B. A dma_gather descriptor fetches one pair-row; the edge's parity
    picks which half feeds the PE (lhsT free-dim slice).
  - Window-major convs with an SBUF accumulator: for each pair-window
    (25088 pair-rows), the producer phase segment for that window is
    emitted first, then the window's edge chunks (gathers on 4 SWDGE
    queues round-robin), so producer and conv overlap. Per (tile,
    window) PSUM run -> DVE add into acc[64, dpad]. A final strip
    epilogue applies dinv[dst] and bias.
  - Phase A (replicated): h1l pair-rows = (dinv*x) @ W1 for ALL nodes.
  - AllGather of h1T shards (bf16) = the halo exchange.
  - Phase C (replicated): h2l pair-rows = h1 @ W2 for ALL nodes.
  - Conv2 -> h2T strips (f32, SBUF) feeding the MLP head per strip.

Host preprocessing is structure-only (derived from edge_index). All
cores share one program: block structure is the max across cores. Pad
slots carry gidx=0 (harmless row) / dstl=-1 (one-hot zeroes them);
trailing pads of each gather call are clipped via num_idxs.
"""

import numpy as np
import ml_dtypes

import concourse.bass as bass
import concourse.bacc as bacc
import concourse.tile as tile
import concourse.mybir as mybir
from concourse.bass_utils import run_bass_kernel_spmd

F32 = mybir.dt.float32
BF16 = mybir.dt.bfloat16
I16 = mybir.dt.int16

NCORES = 8
WIN = 25088          # pair-rows per gather window (< int16 max)
EB = 128             # edges per block (PE contraction height)
DTILE = 64           # dst tile width (one-hot cols, PSUM agg cols)
CB = 8               # max blocks per dma_gather (1024-idx HW limit)
NQ = 4               # SWDGE queues (ucode max)
SG = 16              # chunks per coalesced index-load supergroup
AC = 512             # phase A/C node-chunk
ES = 512             # epilogue/head strip width


def wrap16x8(a):
    """[n] int16 -> [128, n//16]: idx i at [i%16, i//16], replicated x8."""
    w = np.ascontiguousarray(np.transpose(a.reshape(-1, 16), (1, 0)))
    return np.ascontiguousarray(np.tile(w, (8, 1)))


# ----------------------------------------------------------------------------
# host-side preprocessing (numpy only)
# ----------------------------------------------------------------------------

def preprocess(n, edge_index):
    src = edge_index[0].astype(np.int64)
    dst = edge_index[1].astype(np.int64)

    deg = np.bincount(dst, minlength=n).astype(np.float64) + 1.0
    dinv = (1.0 / np.sqrt(deg)).astype(np.float32)

    shard = n // NCORES
    assert shard * NCORES == n and shard % 2 == 0
    ntiles = (shard + DTILE - 1) // DTILE
    dpad = ntiles * DTILE
    npairs = (n + 1) // 2
    nvp = (npairs + WIN - 1) // WIN          # pair windows
    nv = nvp * 2                              # groups: (pair window, parity)

    loops = np.arange(n, dtype=np.int64)
    src = np.concatenate([src, loops])
    dst = np.concatenate([dst, loops])

    # per-core edge lists grouped by (dst tile, group v)
    per_core = []
    counts = np.zeros((NCORES, ntiles, nv), np.int64)
    for c in range(NCORES):
        base = c * shard
        m = (dst >= base) & (dst < base + shard)
        s, d = src[m], dst[m] - base
        t_id = d // DTILE
        v_id = (s // 2) // WIN * 2 + (s % 2)
        order = np.lexsort((v_id, t_id))
        s, d, t_id, v_id = s[order], d[order], t_id[order], v_id[order]
        np.add.at(counts[c], (t_id, v_id), 1)
        per_core.append((s, d, t_id, v_id))

    cmax = counts.max(axis=0)                 # [ntiles, nv]
    nb = (cmax + EB - 1) // EB                # blocks per (t, v)

    # window-major shared structure: for vpw: for t: blocks of v=2vpw,2vpw+1
    chunks = []       # [vpw, k, nidx]
    blocks_raw = []   # (t, par, grp_block_idx)
    seg_start = []    # first chunk index of each vpw segment
    for vpw in range(nvp):
        seg_start.append(len(chunks))
        for t in range(ntiles):
            blk = []
            for par in range(2):
                v = vpw * 2 + par
                for b in range(int(nb[t, v])):
                    blk.append((t, par, b))
            b0 = 0
            while b0 < len(blk):
                k = min(CB, len(blk) - b0)
                chunks.append([vpw, k, 0])
                blocks_raw.extend(blk[b0:b0 + k])
                b0 += k
    seg_start.append(len(chunks))

    # per-chunk num_idxs: last real slot (max over cores), ceil16;
    # blocks fully past it are never executed
    goff, boff = [], []
    execf = []
    g0 = b0_ = 0
    bi = 0
    for ch in chunks:
        vpw, k, _ = ch
        goff.append(g0); boff.append(b0_)
        last = EB  # first block always partially real (structure nonempty)
        lastj = 0
        for j in range(k):
            t, par, b = blocks_raw[bi + j]
            v = vpw * 2 + par
            real = int(min(max(cmax[t, v] - b * EB, 0), EB))
            if real > 0:
                last = j * EB + real
                lastj = j
        nidx = max((last + 15) // 16 * 16, 16)
        ch[2] = nidx
        kk = (nidx + EB - 1) // EB
        execf.extend(j < kk for j in range(k))
        bi += k
        g0 += k * EB // 16
        b0_ += k

    # start/stop per (t, vpw) PSUM run over executed blocks
    vpw_of_block = []
    for ci, (vpw, k, nidx) in enumerate(chunks):
        vpw_of_block.extend([vpw] * k)
    first, last = {}, {}
    for i, ((t, par, b), e) in enumerate(zip(blocks_raw, execf)):
        if e:
            key = (t, vpw_of_block[i])
            first.setdefault(key, i)
            last[key] = i
    blocks = [
        (t, par, e,
         e and first[(t, vpw_of_block[i])] == i,
         e and last[(t, vpw_of_block[i])] == i)
        for i, ((t, par, b), e) in enumerate(zip(blocks_raw, execf))
    ]

    # group slot offsets in the shared layout (walk block list in order)
    grp_off = {}
    slot = 0
    for (t, par, b), vpw in zip(blocks_raw, vpw_of_block):
        key = (t, vpw * 2 + par)
        if key not in grp_off:
            grp_off[key] = slot
        slot += EB

    cores = []
    for c in range(NCORES):
        s, d, t_id, v_id = per_core[c]
        gidx = np.zeros((b0_ * EB,), np.int16)
        dstl = np.full((b0_ * EB,), -1.0, np.float32)
        key = t_id * nv + v_id
        cuts = np.flatnonzero(np.diff(key)) + 1
        starts = np.concatenate([[0], cuts]) if len(s) else np.array([], np.int64)
        ends = np.concatenate([cuts, [len(s)]]) if len(s) else np.array([], np.int64)
        for a, b in zip(starts, ends):
            t = int(t_id[a]); v = int(v_id[a])
            o = grp_off[(t, v)]
            cnt = b - a
            gidx[o:o + cnt] = ((s[a:b] // 2) - (v // 2) * WIN).astype(np.int16)
            dstl[o:o + cnt] = (d[a:b] - t * DTILE).astype(np.float32)
        cores.append(dict(
            gidx=wrap16x8(gidx),
            dstl=np.ascontiguousarray(
                dstl.reshape(b0_, EB).T.astype(ml_dtypes.bfloat16)),
        ))

    plan = dict(chunks=chunks, blocks=blocks, goff=goff, boff=boff,
                seg_start=seg_start, ntiles=ntiles, dpad=dpad, shard=shard,
                nvp=nvp, gcols=max(g0, 16), bcols=max(b0_, 1))
    return dinv, plan, cores


# ----------------------------------------------------------------------------
# device program
# ----------------------------------------------------------------------------

def emit_conv(nc, pools, plan, hbuf, gidx_d, dstl_d, dinvrep_d, iota_t,
              bias_t, phase_cb, strip_cb, conv_id):
    """Window-major conv: phase_cb(vpw) producers, gathers, PSUM runs ->
    SBUF acc; then strip epilogue: strip_cb(o, w, e1_f32)."""
    pool, gpool, ipool, apool, psag = pools
    chunks, blocks = plan["chunks"], plan["blocks"]
    dpad = plan["dpad"]

    acc = apool.tile([64, dpad], F32, tag="acc", name=f"acc{conv_id}")
    nc.vector.memset(acc[:], 0.0)

    agg = {"t": None}
    sup = {"next": 0}
    bi = 0
    for vpw in range(plan["nvp"]):
        phase_cb(vpw)
        for ci in range(plan["seg_start"][vpw], plan["seg_start"][vpw + 1]):
            _, k, nidx = chunks[ci]
            if ci >= sup["next"]:
                hi = min(ci + SG, len(chunks))
                g0 = plan["goff"][ci]
                b0 = plan["boff"][ci]
                g1 = plan["goff"][hi - 1] + chunks[hi - 1][1] * EB // 16
                b1 = plan["boff"][hi - 1] + chunks[hi - 1][1]
                it = ipool.tile([128, SG * CB * EB // 16], I16, tag="gidx")
                nc.sync.dma_start(it[:, :g1 - g0], gidx_d[:, g0:g1])
                dl = ipool.tile([128, SG * CB], BF16, tag="dstl")
                nc.sync.dma_start(dl[:, :b1 - b0], dstl_d[:, b0:b1])
                sup = {"it": it, "dl": dl, "g0": g0, "b0": b0, "next": hi}
            go = plan["goff"][ci] - sup["g0"]
            bo = plan["boff"][ci] - sup["b0"]
            it, dl = sup["it"], sup["dl"]
            kk = (nidx + EB - 1) // EB
            g = gpool.tile([128, CB, 128], BF16, tag="g")
            nc.gpsimd.dma_gather(
                g[:, :kk, :],
                hbuf[vpw * WIN:(vpw + 1) * WIN, :],
                it[:, go:go + nidx // 16],
                num_idxs=nidx, num_idxs_reg=nidx, elem_size=128,
                queue_num=ci % NQ,
            )
            s_t = gpool.tile([128, CB, DTILE], BF16, tag="s")
            nc.vector.tensor_tensor(
                s_t[:, :k, :],
                iota_t[:].unsqueeze(1).broadcast_to([128, k, DTILE]),
                dl[:, bo:bo + k].unsqueeze(2).broadcast_to([128, k, DTILE]),
                op=mybir.AluOpType.is_equal,
            )
            for j in range(k):
                t_, half, ex, start, stop = blocks[bi]; bi += 1
                if not ex:
                    continue
                if start:
                    agg["t"] = psag.tile([64, DTILE], F32, tag="agg",
                                         name=f"agg{conv_id}_{bi}")
                nc.tensor.matmul(agg["t"][:],
                                 lhsT=g[:, j, half * 64:(half + 1) * 64],
                                 rhs=s_t[:, j, :], start=start, stop=stop)
                if stop:
                    sl = acc[:, t_ * DTILE:(t_ + 1) * DTILE]
                    nc.vector.tensor_tensor(sl, sl, agg["t"][:],
                                            op=mybir.AluOpType.add)

    # strip epilogue: e1 = acc * dinv[dst]
    for o in range(0, dpad, ES):
        w_ = min(ES, dpad - o)
        dr = pool.tile([64, ES], F32, tag="dr")
        nc.sync.dma_start(dr[:, :w_], dinvrep_d[:, o:o + w_])
        e1 = pool.tile([64, ES], F32, tag="ep1")
        nc.vector.tensor_tensor(e1[:, :w_], acc[:, o:o + w_], dr[:, :w_],
                                op=mybir.AluOpType.mult)
        strip_cb(o, w_, e1)


def build_program(meta, plan):
    npad = meta["npad"]
    dpad = plan["dpad"]
    shard = plan["shard"]
    nptiles = dpad // 128
    gcols = plan["gcols"]
    bcols = plan["bcols"]

    nc = bacc.Bacc("TRN2", target_bir_lowering=False, debug=False,
                   num_devices=NCORES, num_swdge_queues=NQ)

    xt = nc.dram_tensor("xt", [128, npad], BF16, kind="ExternalInput")
    h1buf = nc.dram_tensor("h1buf", [npad // 2, 128], BF16,
                           kind="ExternalInput")
    h2buf = nc.dram_tensor("h2buf", [npad // 2, 128], BF16,
                           kind="ExternalInput")
    gidx_d = nc.dram_tensor("gidx", [128, gcols], I16, kind="ExternalInput")
    dstl_d = nc.dram_tensor("dstl", [128, bcols], BF16, kind="ExternalInput")
    w1_d = nc.dram_tensor("w1", [128, 64], BF16, kind="ExternalInput")
    w2_d = nc.dram_tensor("w2", [64, 64], BF16, kind="ExternalInput")
    lw1_d = nc.dram_tensor("lw1", [64, 64], F32, kind="ExternalInput")
    lw2_d = nc.dram_tensor("lw2", [64, 32], F32, kind="ExternalInput")
    lw3_d = nc.dram_tensor("lw3", [32, 1], F32, kind="ExternalInput")
    b1_d = nc.dram_tensor("b1", [64, 1], F32, kind="ExternalInput")
    b2_d = nc.dram_tensor("b2", [64, 1], F32, kind="ExternalInput")
    lb1_d = nc.dram_tensor("lb1", [64, 1], F32, kind="ExternalInput")
    lb2_d = nc.dram_tensor("lb2", [32, 1], F32, kind="ExternalInput")
    lb3_d = nc.dram_tensor("lb3", [1, 1], F32, kind="ExternalInput")
    iota_d = nc.dram_tensor("iota", [128, DTILE], BF16, kind="ExternalInput")
    dinvrep_d = nc.dram_tensor("dinvrep", [64, dpad], F32, kind="ExternalInput")
    dinvc_d = nc.dram_tensor("dinvc", [128, NCORES * nptiles], F32,
                             kind="ExternalInput")
    out_d = nc.dram_tensor("out", [1, dpad], F32, kind="ExternalOutput")

    with tile.TileContext(nc) as tc:
        with (
            tc.tile_pool(name="const", bufs=1) as cpool,
            tc.tile_pool(name="accp", bufs=1) as apool,
            tc.tile_pool(name="work", bufs=6) as pool,
            tc.tile_pool(name="gat", bufs=10) as gpool,
            tc.tile_pool(name="head", bufs=3) as hpool,
            tc.tile_pool(name="idx", bufs=3) as ipool,
            tc.tile_pool(name="xtp", bufs=3) as xtpool,
            tc.tile_pool(name="psag", bufs=3, space="PSUM") as psag,
            tc.tile_pool(name="psmm", bufs=2, space="PSUM") as psmm,
            tc.tile_pool(name="pshd", bufs=1, space="PSUM") as pshd,
            tc.tile_pool(name="dram", bufs=1, space="DRAM") as dram,
        ):
            def load_const(dram_t, shape, dtype, tag):
                t = cpool.tile(shape, dtype, tag=tag)
                nc.sync.dma_start(t[:], dram_t[:])
                return t

            w1_t = load_const(w1_d, [128, 64], BF16, "w1")
            w2_t = load_const(w2_d, [64, 64], BF16, "w2")
            lw1_t = load_const(lw1_d, [64, 64], F32, "lw1")
            lw2_t = load_const(lw2_d, [64, 32], F32, "lw2")
            lw3_t = load_const(lw3_d, [32, 1], F32, "lw3")
            b1_t = load_const(b1_d, [64, 1], F32, "b1")
            b2_t = load_const(b2_d, [64, 1], F32, "b2")
            lb1_t = load_const(lb1_d, [64, 1], F32, "lb1")
            lb2_t = load_const(lb2_d, [32, 1], F32, "lb2")
            lb3_t = load_const(lb3_d, [1, 1], F32, "lb3")
            iota_t = load_const(iota_d, [128, DTILE], BF16, "iota")
            dinvc_t = load_const(dinvc_d, [128, NCORES * nptiles], F32, "dinvc")

            eng = [nc.scalar, nc.gpsimd, nc.sync]
            pools = (pool, gpool, ipool, apool, psag)

            for _mi in range(10):
                gz = gpool.tile([128, CB, 128], BF16, tag="g", name=f"gz{_mi}")
                nc.vector.memset(gz[:], 0.0)

            # ---------------- conv1 (phase A interleaved) ----------------
            def phase_a(vpw):
                base = vpw * 2 * WIN
                for t in range(2 * WIN // AC):
                    col = base + t * AC
                    st = xtpool.tile([128, AC], BF16, tag="xt")
                    eng[t % 3].dma_start(st[:], xt[:, col:col + AC])
                    for j in range(AC // 128):
                        ps = psmm.tile([128, 64], F32, tag="mm")
                        nc.tensor.matmul(
                            ps[:], lhsT=st[:, j * 128:(j + 1) * 128],
                            rhs=w1_t[:], start=True, stop=True)
                        sb = pool.tile([128, 64], BF16, tag="arow")
                        if j % 2 == 0:
                            nc.vector.tensor_copy(sb[:], ps[:])
                        else:
                            nc.scalar.activation(
                                sb[:], ps[:],
                                mybir.ActivationFunctionType.Copy)
                        r = col + j * 128
                        eng[(t + j + 1) % 3].dma_start(
                            h1buf[r // 2:r // 2 + 64, :], sb[:])

            h1t_bounce = dram.tile([64, dpad], BF16)
            ag_out = dram.tile([NCORES * 64, dpad], BF16, addr_space="Shared")

            def conv1_strip(o, w_, e1):
                e2 = pool.tile([64, ES], BF16, tag="ep2")
                nc.scalar.activation(e2[:, :w_], e1[:, :w_],
                                     mybir.ActivationFunctionType.Identity,
                                     bias=b1_t[:])
                nc.scalar.dma_start(h1t_bounce[:, o:o + w_], e2[:, :w_])

            emit_conv(nc, pools, plan, h1buf, gidx_d, dstl_d, dinvrep_d,
                      iota_t, b1_t, phase_a, conv1_strip, 1)

            if dpad > shard:
                zt = pool.tile([64, dpad - shard], BF16, tag="zt")
                nc.vector.memset(zt[:], 0.0)
                nc.sync.dma_start(h1t_bounce[:, shard:], zt[:])

            nc.gpsimd.collective_compute(
                "AllGather", mybir.AluOpType.bypass,
                ins=[h1t_bounce[:].opt()],
                outs=[ag_out[:].opt()],
                replica_groups=[list(range(NCORES))],
            )

            # ---------------- conv2 (phase C interleaved) ----------------
            def phase_c_range(items):
                for ii, (c, o) in enumerate(items):
                    cw = min(AC, shard - o)
                    st = xtpool.tile([64, AC], BF16, tag="ct")
                    eng[ii % 3].dma_start(
                        st[:, :cw], ag_out[c * 64:(c + 1) * 64, o:o + cw])
                    for j in range(0, cw, 128):
                        cnt = min(128, cw - j)
                        n0 = c * shard + o + j
                        t = (o + j) // 128
                        ps = psmm.tile([128, 64], F32, tag="mm")
                        nc.tensor.matmul(ps[:cnt, :], lhsT=st[:, j:j + cnt],
                                         rhs=w2_t[:], start=True, stop=True)
                        sb = pool.tile([128, 64], BF16, tag="crow")
                        dslice = dinvc_t[:cnt,
                                         c * nptiles + t:c * nptiles + t + 1]
                        if (j // 128) % 2 == 0:
                            nc.vector.tensor_tensor(
                                sb[:cnt, :], ps[:cnt, :],
                                dslice.broadcast_to([cnt, 64]),
                                op=mybir.AluOpType.mult)
                        else:
                            nc.scalar.activation(
                                sb[:cnt, :], ps[:cnt, :],
                                mybir.ActivationFunctionType.Copy,
                                scale=dslice)
                        eng[(j // 128 + 1) % 3].dma_start(
                            h2buf[n0 // 2:(n0 + cnt) // 2, :], sb[:cnt, :])

            all_c = [(c, o) for c in range(NCORES)
                     for o in range(0, shard, AC)]
            cut = 0
            while cut < len(all_c):
                c, o = all_c[cut]
                if c * shard + o >= 2 * WIN:
                    break
                cut += 1
            cut = min(cut + 1, len(all_c))

            def phase_c(vpw):
                phase_c_range(all_c[:cut] if vpw == 0 else all_c[cut:])

            EC = ES
            nhc = (dpad + EC - 1) // EC
            h2t_tiles = [
                cpool.tile([64, min(EC, dpad - o * EC)], F32, tag=f"h2t{o}",
                           name=f"h2t{o}")
                for o in range(nhc)
            ]

            def conv2_strip(o, w_, e1):
                nc.scalar.activation(h2t_tiles[o // EC][:, :w_], e1[:, :w_],
                                     mybir.ActivationFunctionType.Identity,
                                     bias=b2_t[:])

            emit_conv(nc, pools, plan, h2buf, gidx_d, dstl_d, dinvrep_d,
                      iota_t, b2_t, phase_c, conv2_strip, 2)

            # ---------------- MLP head (transposed space) ----------------
            for hc in range(nhc):
                o = hc * EC
                w_ = min(EC, dpad - o)
                p1 = pshd.tile([64, EC], F32, tag="mm1")
                nc.tensor.matmul(p1[:, :w_], lhsT=lw1_t[:],
                                 rhs=h2t_tiles[hc][:, :w_], start=True,
                                 stop=True)
                z1 = hpool.tile([64, EC], F32, tag="z1")
                nc.scalar.activation(z1[:, :w_], p1[:, :w_],
                                     mybir.ActivationFunctionType.Relu,
                                     bias=lb1_t[:])
                p2 = pshd.tile([32, EC], F32, tag="mm2")
                nc.tensor.matmul(p2[:, :w_], lhsT=lw2_t[:], rhs=z1[:, :w_],
                                 start=True, stop=True)
                z2 = hpool.tile([32, EC], F32, tag="z2")
                nc.scalar.activation(z2[:, :w_], p2[:, :w_],
                                     mybir.ActivationFunctionType.Relu,
                                     bias=lb2_t[:])
                p3 = pshd.tile([1, EC], F32, tag="mm3")
                nc.tensor.matmul(p3[:, :w_], lhsT=lw3_t[:], rhs=z2[:, :w_],
                                 start=True, stop=True)
                z3 = hpool.tile([1, EC], F32, tag="z3")
                nc.vector.tensor_tensor(z3[:, :w_], p3[:, :w_],
                                        lb3_t[:].broadcast_to([1, w_]),
                                        op=mybir.AluOpType.add)
                nc.sync.dma_start(out_d[:, o:o + w_], z3[:, :w_])

    nc.compile()
    return nc


# ----------------------------------------------------------------------------
# entry point
# ----------------------------------------------------------------------------

def kernel(x, edge_index, W1, b1, W2, b2, lw1, lb1, lw2, lb2, lw3, lb3,
           _want_trace=False):
    x = np.asarray(x, np.float32)
    edge_index = np.asarray(edge_index)
    n = x.shape[0]
    npad = ((n // 2 + WIN - 1) // WIN) * WIN * 2

    dinv, plan, cores = preprocess(n, edge_index)
    shard, dpad = plan["shard"], plan["dpad"]
    nptiles = dpad // 128

    xt = np.zeros((128, npad), ml_dtypes.bfloat16)
    xt[:, :n] = (x * dinv[:, None]).T.astype(ml_dtypes.bfloat16)
    hz = np.zeros((npad // 2, 128), ml_dtypes.bfloat16)
    iota = np.tile(np.arange(DTILE, dtype=np.float32), (128, 1)).astype(
        ml_dtypes.bfloat16)

    dinvc = np.zeros((128, NCORES * nptiles), np.float32)
    for cc in range(NCORES):
        for t in range(nptiles):
            n0 = cc * shard + t * 128
            cnt = min(128, (cc + 1) * shard - n0)
            dinvc[:cnt, cc * nptiles + t] = dinv[n0:n0 + cnt]

    in_maps = []
    for c in range(NCORES):
        dinvrep = np.zeros((64, dpad), np.float32)
        dinvrep[:, :shard] = dinv[c * shard:(c + 1) * shard][None, :]
        in_maps.append({
            "xt": xt, "h1buf": hz, "h2buf": hz,
            "gidx": cores[c]["gidx"], "dstl": cores[c]["dstl"],
            "w1": np.asarray(W1, np.float32).astype(ml_dtypes.bfloat16),
            "w2": np.asarray(W2, np.float32).astype(ml_dtypes.bfloat16),
            "lw1": np.ascontiguousarray(np.asarray(lw1, np.float32)),
            "lw2": np.ascontiguousarray(np.asarray(lw2, np.float32)),
            "lw3": np.ascontiguousarray(np.asarray(lw3, np.float32)),
            "b1": np.asarray(b1, np.float32).reshape(-1, 1),
            "b2": np.asarray(b2, np.float32).reshape(-1, 1),
            "lb1": np.asarray(lb1, np.float32).reshape(-1, 1),
            "lb2": np.asarray(lb2, np.float32).reshape(-1, 1),
            "lb3": np.asarray(lb3, np.float32).reshape(-1, 1),
            "iota": iota, "dinvrep": dinvrep, "dinvc": dinvc,
        })

    meta = {"n": n, "npad": npad}
    nc = build_program(meta, plan)

    res = run_bass_kernel_spmd(nc, in_maps, core_ids=list(range(NCORES)),
                               trace=_want_trace)
    out = np.empty((n, 1), np.float32)
    for c in range(NCORES):
        out[c * shard:(c + 1) * shard, 0] = res.results[c]["out"][0, :shard]
    kernel._last_exec_ns = res.exec_time_ns
    return out


# revision 18
# speedup vs baseline: 1.0277x; 1.0277x over previous
"""GCN (2x GCNConv + MLP head) on 8 TRN2 NeuronCores via Bass/Tile.

Distribution (graph-parallel, per the node-sharding scheme):
  - nodes sharded by id across 8 cores (12500 each); weights replicated.
  - h buffers are bf16 pair-rows [npad/2, 128]: row r = nodes (2r, 2r+1),
    256B. A dma_gather descriptor fetches one pair-row; the edge's parity
    picks which half feeds the PE (lhsT free-dim slice).
  - Window-major convs with an SBUF accumulator: for each pair-window
    (25088 pair-rows), the producer phase segment for that window is
    emitted first, then the window's edge chunks (gathers on 4 SWDGE
    queues round-robin), so producer and conv overlap. Per (tile,
    window) PSUM run -> DVE add into acc[64, dpad]. A final strip
    epilogue applies dinv[dst] and bias.
  - Phase A (replicated): h1l pair-rows = (dinv*x) @ W1 for ALL nodes.
  - AllGather of h1T shards (bf16) = the halo exchange.
  - Phase C (replicated): h2l pair-rows = h1 @ W2 for ALL nodes.
  - Conv2 -> h2T strips (f32, SBUF) feeding the MLP head per strip.

Host preprocessing is structure-only (derived from edge_index). All
cores share one program: block structure is the max across cores. Pad
slots carry gidx=0 (harmless row) / dstl=-1 (one-hot zeroes them);
trailing pads of each gather call are clipped via num_idxs.
"""

import numpy as np
import ml_dtypes

import concourse.bass as bass
import concourse.bacc as bacc
import concourse.tile as tile
import concourse.mybir as mybir
from concourse.bass_utils import run_bass_kernel_spmd

F32 = mybir.dt.float32
BF16 = mybir.dt.bfloat16
I16 = mybir.dt.int16

NCORES = 8
WIN = 25088          # pair-rows per gather window (< int16 max)
EB = 128             # edges per block (PE contraction height)
DTILE = 64           # dst tile width (one-hot cols, PSUM agg cols)
CB = 8               # max blocks per dma_gather (1024-idx HW limit)
NQ = 4               # SWDGE queues (ucode max)
SG = 16              # chunks per coalesced index-load supergroup
AC = 512             # phase A/C node-chunk
ES = 512             # epilogue/head strip width


def wrap16x8(a):
    """[n] int16 -> [128, n//16]: idx i at [i%16, i//16], replicated x8."""
    w = np.ascontiguousarray(np.transpose(a.reshape(-1, 16), (1, 0)))
    return np.ascontiguousarray(np.tile(w, (8, 1)))


# ----------------------------------------------------------------------------
# host-side preprocessing (numpy only)
# ----------------------------------------------------------------------------

def preprocess(n, edge_index):
    src = edge_index[0].astype(np.int64)
    dst = edge_index[1].astype(np.int64)

    deg = np.bincount(dst, minlength=n).astype(np.float64) + 1.0
    dinv = (1.0 / np.sqrt(deg)).astype(np.float32)

    shard = n // NCORES
    assert shard * NCORES == n and shard % 2 == 0
    ntiles = (shard + DTILE - 1) // DTILE
    dpad = ntiles * DTILE
    npairs = (n + 1) // 2
    nvp = (npairs + WIN - 1) // WIN          # pair windows
    nv = nvp * 2                              # groups: (pair window, parity)

    loops = np.arange(n, dtype=np.int64)
    src = np.concatenate([src, loops])
    dst = np.concatenate([dst, loops])

    # per-core edge lists grouped by (dst tile, group v)
    per_core = []
    counts = np.zeros((NCORES, ntiles, nv), np.int64)
    for c in range(NCORES):
        base = c * shard
        m = (dst >= base) & (dst < base + shard)
        s, d = src[m], dst[m] - base
        t_id = d // DTILE
        v_id = (s // 2) // WIN * 2 + (s % 2)
        order = np.lexsort((v_id, t_id))
        s, d, t_id, v_id = s[order], d[order], t_id[order], v_id[order]
        np.add.at(counts[c], (t_id, v_id), 1)
        per_core.append((s, d, t_id, v_id))

    cmax = counts.max(axis=0)                 # [ntiles, nv]
    nb = (cmax + EB - 1) // EB                # blocks per (t, v)

    # window-major shared structure: for vpw: for t: blocks of v=2vpw,2vpw+1
    chunks = []       # [vpw, k, nidx]
    blocks_raw = []   # (t, par, grp_block_idx)
    seg_start = []    # first chunk index of each vpw segment
    for vpw in range(nvp):
        seg_start.append(len(chunks))
        for t in range(ntiles):
            blk = []
            for par in range(2):
                v = vpw * 2 + par
                for b in range(int(nb[t, v])):
                    blk.append((t, par, b))
            b0 = 0
            while b0 < len(blk):
                k = min(CB, len(blk) - b0)
                chunks.append([vpw, k, 0])
                blocks_raw.extend(blk[b0:b0 + k])
                b0 += k
    seg_start.append(len(chunks))

    # per-chunk num_idxs: last real slot (max over cores), ceil16;
    # blocks fully past it are never executed
    goff, boff = [], []
    execf = []
    g0 = b0_ = 0
    bi = 0
    for ch in chunks:
        vpw, k, _ = ch
        goff.append(g0); boff.append(b0_)
        last = EB  # first block always partially real (structure nonempty)
        lastj = 0
        for j in range(k):
            t, par, b = blocks_raw[bi + j]
            v = vpw * 2 + par
            real = int(min(max(cmax[t, v] - b * EB, 0), EB))
            if real > 0:
                last = j * EB + real
                lastj = j
        nidx = max((last + 15) // 16 * 16, 16)
        ch[2] = nidx
        kk = (nidx + EB - 1) // EB
        execf.extend(j < kk for j in range(k))
        bi += k
        g0 += k * EB // 16
        b0_ += k

    # start/stop per (t, vpw) PSUM run over executed blocks
    vpw_of_block = []
    for ci, (vpw, k, nidx) in enumerate(chunks):
        vpw_of_block.extend([vpw] * k)
    first, last = {}, {}
    for i, ((t, par, b), e) in enumerate(zip(blocks_raw, execf)):
        if e:
            key = (t, vpw_of_block[i])
            first.setdefault(key, i)
            last[key] = i
    blocks = [
        (t, par, e,
         e and first[(t, vpw_of_block[i])] == i,
         e and last[(t, vpw_of_block[i])] == i)
        for i, ((t, par, b), e) in enumerate(zip(blocks_raw, execf))
    ]

    # group slot offsets in the shared layout (walk block list in order)
    grp_off = {}
    slot = 0
    for (t, par, b), vpw in zip(blocks_raw, vpw_of_block):
        key = (t, vpw * 2 + par)
        if key not in grp_off:
            grp_off[key] = slot
        slot += EB

    cores = []
    for c in range(NCORES):
        s, d, t_id, v_id = per_core[c]
        gidx = np.zeros((b0_ * EB,), np.int16)
        dstl = np.full((b0_ * EB,), -1.0, np.float32)
        key = t_id * nv + v_id
        cuts = np.flatnonzero(np.diff(key)) + 1
        starts = np.concatenate([[0], cuts]) if len(s) else np.array([], np.int64)
        ends = np.concatenate([cuts, [len(s)]]) if len(s) else np.array([], np.int64)
        for a, b in zip(starts, ends):
            t = int(t_id[a]); v = int(v_id[a])
            o = grp_off[(t, v)]
            cnt = b - a
            gidx[o:o + cnt] = ((s[a:b] // 2) - (v // 2) * WIN).astype(np.int16)
            dstl[o:o + cnt] = (d[a:b] - t * DTILE).astype(np.float32)
        cores.append(dict(
            gidx=wrap16x8(gidx),
            dstl=np.ascontiguousarray(
                dstl.reshape(b0_, EB).T.astype(ml_dtypes.bfloat16)),
        ))

    plan = dict(chunks=chunks, blocks=blocks, goff=goff, boff=boff,
                seg_start=seg_start, ntiles=ntiles, dpad=dpad, shard=shard,
                nvp=nvp, gcols=max(g0, 16), bcols=max(b0_, 1))
    return dinv, plan, cores


# ----------------------------------------------------------------------------
# device program
# ----------------------------------------------------------------------------

def emit_conv(nc, pools, plan, hbuf, gidx_d, dstl_d, dinvrep_d, iota_t,
              bias_t, phase_cb, strip_cb, conv_id):
    """Window-major conv: phase_cb(vpw) producers, gathers, PSUM runs ->
    SBUF acc; then strip epilogue: strip_cb(o, w, e1_f32)."""
    pool, gpool, ipool, apool, psag = pools
    chunks, blocks = plan["chunks"], plan["blocks"]
    dpad = plan["dpad"]

    acc = apool.tile([64, dpad], F32, tag="acc", name=f"acc{conv_id}")
    nc.vector.memset(acc[:], 0.0)

    agg = {"t": None}
    sup = {"next": 0}
    bi = 0
    for vpw in range(plan["nvp"]):
        phase_cb(vpw)
        for ci in range(plan["seg_start"][vpw], plan["seg_start"][vpw + 1]):
            _, k, nidx = chunks[ci]
            if ci >= sup["next"]:
                hi = min(ci + SG, len(chunks))
                g0 = plan["goff"][ci]
                b0 = plan["boff"][ci]
                g1 = plan["goff"][hi - 1] + chunks[hi - 1][1] * EB // 16
                b1 = plan["boff"][hi - 1] + chunks[hi - 1][1]
                it = ipool.tile([128, SG * CB * EB // 16], I16, tag="gidx")
                nc.sync.dma_start(it[:, :g1 - g0], gidx_d[:, g0:g1])
                dl = ipool.tile([128, SG * CB], BF16, tag="dstl")
                nc.sync.dma_start(dl[:, :b1 - b0], dstl_d[:, b0:b1])
                sup = {"it": it, "dl": dl, "g0": g0, "b0": b0, "next": hi}
            go = plan["goff"][ci] - sup["g0"]
            bo = plan["boff"][ci] - sup["b0"]
            it, dl = sup["it"], sup["dl"]
            kk = (nidx + EB - 1) // EB
            g = gpool.tile([128, CB, 128], BF16, tag="g")
            nc.gpsimd.dma_gather(
                g[:, :kk, :],
                hbuf[vpw * WIN:(vpw + 1) * WIN, :],
                it[:, go:go + nidx // 16],
                num_idxs=nidx, num_idxs_reg=nidx, elem_size=128,
                queue_num=ci % NQ,
            )
            s_t = gpool.tile([128, CB, DTILE], BF16, tag="s")
            nc.vector.tensor_tensor(
                s_t[:, :k, :],
                iota_t[:].unsqueeze(1).broadcast_to([128, k, DTILE]),
                dl[:, bo:bo + k].unsqueeze(2).broadcast_to([128, k, DTILE]),
                op=mybir.AluOpType.is_equal,
            )
            for j in range(k):
                t_, half, ex, start, stop = blocks[bi]; bi += 1
                if not ex:
                    continue
                if start:
                    agg["t"] = psag.tile([64, DTILE], F32, tag="agg",
                                         name=f"agg{conv_id}_{bi}")
                nc.tensor.matmul(agg["t"][:],
                                 lhsT=g[:, j, half * 64:(half + 1) * 64],
                                 rhs=s_t[:, j, :], start=start, stop=stop)
                if stop:
                    sl = acc[:, t_ * DTILE:(t_ + 1) * DTILE]
                    nc.vector.tensor_tensor(sl, sl, agg["t"][:],
                                            op=mybir.AluOpType.add)

    # strip epilogue: e1 = acc * dinv[dst]
    for o in range(0, dpad, ES):
        w_ = min(ES, dpad - o)
        dr = pool.tile([64, ES], F32, tag="dr")
        nc.sync.dma_start(dr[:, :w_], dinvrep_d[:, o:o + w_])
        e1 = pool.tile([64, ES], F32, tag="ep1")
        nc.vector.tensor_tensor(e1[:, :w_], acc[:, o:o + w_], dr[:, :w_],
                                op=mybir.AluOpType.mult)
        strip_cb(o, w_, e1)


def build_program(meta, plan):
    npad = meta["npad"]
    dpad = plan["dpad"]
    shard = plan["shard"]
    nptiles = dpad // 128
    gcols = plan["gcols"]
    bcols = plan["bcols"]

    nc = bacc.Bacc("TRN2", target_bir_lowering=False, debug=False,
                   num_devices=NCORES, num_swdge_queues=NQ)

    xt = nc.dram_tensor("xt", [128, npad], BF16, kind="ExternalInput")
    h1buf = nc.dram_tensor("h1buf", [npad // 2, 128], BF16,
                           kind="ExternalInput")
    h2buf = nc.dram_tensor("h2buf", [npad // 2, 128], BF16,
                           kind="ExternalInput")
    gidx_d = nc.dram_tensor("gidx", [128, gcols], I16, kind="ExternalInput")
    dstl_d = nc.dram_tensor("dstl", [128, bcols], BF16, kind="ExternalInput")
    w1_d = nc.dram_tensor("w1", [128, 64], BF16, kind="ExternalInput")
    w2_d = nc.dram_tensor("w2", [64, 64], BF16, kind="ExternalInput")
    lw1_d = nc.dram_tensor("lw1", [64, 64], F32, kind="ExternalInput")
    lw2_d = nc.dram_tensor("lw2", [64, 32], F32, kind="ExternalInput")
    lw3_d = nc.dram_tensor("lw3", [32, 1], F32, kind="ExternalInput")
    b1_d = nc.dram_tensor("b1", [64, 1], F32, kind="ExternalInput")
    b2_d = nc.dram_tensor("b2", [64, 1], F32, kind="ExternalInput")
    lb1_d = nc.dram_tensor("lb1", [64, 1], F32, kind="ExternalInput")
    lb2_d = nc.dram_tensor("lb2", [32, 1], F32, kind="ExternalInput")
    lb3_d = nc.dram_tensor("lb3", [1, 1], F32, kind="ExternalInput")
    iota_d = nc.dram_tensor("iota", [128, DTILE], BF16, kind="ExternalInput")
    dinvrep_d = nc.dram_tensor("dinvrep", [64, dpad], F32, kind="ExternalInput")
    dinvc_d = nc.dram_tensor("dinvc", [128, NCORES * nptiles], F32,
                             kind="ExternalInput")
    out_d = nc.dram_tensor("out", [1, dpad], F32, kind="ExternalOutput")

    with tile.TileContext(nc) as tc:
        with (
            tc.tile_pool(name="const", bufs=1) as cpool,
            tc.tile_pool(name="accp", bufs=1) as apool,
            tc.tile_pool(name="work", bufs=6) as pool,
            tc.tile_pool(name="gat", bufs=10) as gpool,
            tc.tile_pool(name="head", bufs=3) as hpool,
            tc.tile_pool(name="idx", bufs=3) as ipool,
            tc.tile_pool(name="xtp", bufs=3) as xtpool,
            tc.tile_pool(name="psag", bufs=3, space="PSUM") as psag,
            tc.tile_pool(name="psmm", bufs=2, space="PSUM") as psmm,
            tc.tile_pool(name="pshd", bufs=1, space="PSUM") as pshd,
            tc.tile_pool(name="dram", bufs=1, space="DRAM") as dram,
        ):
            def load_const(dram_t, shape, dtype, tag):
                t = cpool.tile(shape, dtype, tag=tag)
                nc.sync.dma_start(t[:], dram_t[:])
                return t

            w1_t = load_const(w1_d, [128, 64], BF16, "w1")
            w2_t = load_const(w2_d, [64, 64], BF16, "w2")
            lw1_t = load_const(lw1_d, [64, 64], F32, "lw1")
            lw2_t = load_const(lw2_d, [64, 32], F32, "lw2")
            lw3_t = load_const(lw3_d, [32, 1], F32, "lw3")
            b1_t = load_const(b1_d, [64, 1], F32, "b1")
            b2_t = load_const(b2_d, [64, 1], F32, "b2")
            lb1_t = load_const(lb1_d, [64, 1], F32, "lb1")
            lb2_t = load_const(lb2_d, [32, 1], F32, "lb2")
            lb3_t = load_const(lb3_d, [1, 1], F32, "lb3")
            iota_t = load_const(iota_d, [128, DTILE], BF16, "iota")
            dinvc_t = load_const(dinvc_d, [128, NCORES * nptiles], F32, "dinvc")

            eng = [nc.scalar, nc.sync]
            pools = (pool, gpool, ipool, apool, psag)

            for _mi in range(10):
                gz = gpool.tile([128, CB, 128], BF16, tag="g", name=f"gz{_mi}")
                nc.vector.memset(gz[:], 0.0)

            # ---------------- conv1 (phase A interleaved) ----------------
            def phase_a(vpw):
                base = vpw * 2 * WIN
                for t in range(2 * WIN // AC):
                    col = base + t * AC
                    st = xtpool.tile([128, AC], BF16, tag="xt")
                    eng[t % 2].dma_start(st[:], xt[:, col:col + AC])
                    for j in range(AC // 128):
                        ps = psmm.tile([128, 64], F32, tag="mm")
                        nc.tensor.matmul(
                            ps[:], lhsT=st[:, j * 128:(j + 1) * 128],
                            rhs=w1_t[:], start=True, stop=True)
                        sb = pool.tile([128, 64], BF16, tag="arow")
                        if j % 2 == 0:
                            nc.vector.tensor_copy(sb[:], ps[:])
                        else:
                            nc.scalar.activation(
                                sb[:], ps[:],
                                mybir.ActivationFunctionType.Copy)
                        r = col + j * 128
                        eng[(t + j + 1) % 2].dma_start(
                            h1buf[r // 2:r // 2 + 64, :], sb[:])

            h1t_bounce = dram.tile([64, dpad], BF16)
            ag_out = dram.tile([NCORES * 64, dpad], BF16, addr_space="Shared")

            def conv1_strip(o, w_, e1):
                e2 = pool.tile([64, ES], BF16, tag="ep2")
                nc.scalar.activation(e2[:, :w_], e1[:, :w_],
                                     mybir.ActivationFunctionType.Identity,
                                     bias=b1_t[:])
                nc.scalar.dma_start(h1t_bounce[:, o:o + w_], e2[:, :w_])

            emit_conv(nc, pools, plan, h1buf, gidx_d, dstl_d, dinvrep_d,
                      iota_t, b1_t, phase_a, conv1_strip, 1)

            if dpad > shard:
                zt = pool.tile([64, dpad - shard], BF16, tag="zt")
                nc.vector.memset(zt[:], 0.0)
                nc.sync.dma_start(h1t_bounce[:, shard:], zt[:])

            nc.gpsimd.collective_compute(
                "AllGather", mybir.AluOpType.bypass,
                ins=[h1t_bounce[:].opt()],
                outs=[ag_out[:].opt()],
                replica_groups=[list(range(NCORES))],
            )

            # ---------------- conv2 (phase C interleaved) ----------------
            def phase_c_range(items):
                for ii, (c, o) in enumerate(items):
                    cw = min(AC, shard - o)
                    st = xtpool.tile([64, AC], BF16, tag="ct")
                    eng[ii % 2].dma_start(
                        st[:, :cw], ag_out[c * 64:(c + 1) * 64, o:o + cw])
                    for j in range(0, cw, 128):
                        cnt = min(128, cw - j)
                        n0 = c * shard + o + j
                        t = (o + j) // 128
                        ps = psmm.tile([128, 64], F32, tag="mm")
                        nc.tensor.matmul(ps[:cnt, :], lhsT=st[:, j:j + cnt],
                                         rhs=w2_t[:], start=True, stop=True)
                        sb = pool.tile([128, 64], BF16, tag="crow")
                        dslice = dinvc_t[:cnt,
                                         c * nptiles + t:c * nptiles + t + 1]
                        if (j // 128) % 2 == 0:
                            nc.vector.tensor_tensor(
                                sb[:cnt, :], ps[:cnt, :],
                                dslice.broadcast_to([cnt, 64]),
                                op=mybir.AluOpType.mult)
                        else:
                            nc.scalar.activation(
                                sb[:cnt, :], ps[:cnt, :],
                                mybir.ActivationFunctionType.Copy,
                                scale=dslice)
                        eng[(j // 128 + 1) % 2].dma_start(
                            h2buf[n0 // 2:(n0 + cnt) // 2, :], sb[:cnt, :])

            all_c = [(c, o) for c in range(NCORES)
                     for o in range(0, shard, AC)]
            cut = 0
            while cut < len(all_c):
                c, o = all_c[cut]
                if c * shard + o >= 2 * WIN:
                    break
                cut += 1
            cut = min(cut + 1, len(all_c))

            def phase_c(vpw):
                phase_c_range(all_c[:cut] if vpw == 0 else all_c[cut:])

            EC = ES
            nhc = (dpad + EC - 1) // EC
            h2t_tiles = [
                cpool.tile([64, min(EC, dpad - o * EC)], F32, tag=f"h2t{o}",
                           name=f"h2t{o}")
                for o in range(nhc)
            ]

            def conv2_strip(o, w_, e1):
                nc.scalar.activation(h2t_tiles[o // EC][:, :w_], e1[:, :w_],
                                     mybir.ActivationFunctionType.Identity,
                                     bias=b2_t[:])

            emit_conv(nc, pools, plan, h2buf, gidx_d, dstl_d, dinvrep_d,
                      iota_t, b2_t, phase_c, conv2_strip, 2)

            # ---------------- MLP head (transposed space) ----------------
            for hc in range(nhc):
                o = hc * EC
                w_ = min(EC, dpad - o)
                p1 = pshd.tile([64, EC], F32, tag="mm1")
                nc.tensor.matmul(p1[:, :w_], lhsT=lw1_t[:],
                                 rhs=h2t_tiles[hc][:, :w_], start=True,
                                 stop=True)
                z1 = hpool.tile([64, EC], F32, tag="z1")
                nc.scalar.activation(z1[:, :w_], p1[:, :w_],
                                     mybir.ActivationFunctionType.Relu,
                                     bias=lb1_t[:])
                p2 = pshd.tile([32, EC], F32, tag="mm2")
                nc.tensor.matmul(p2[:, :w_], lhsT=lw2_t[:], rhs=z1[:, :w_],
                                 start=True, stop=True)
                z2 = hpool.tile([32, EC], F32, tag="z2")
                nc.scalar.activation(z2[:, :w_], p2[:, :w_],
                                     mybir.ActivationFunctionType.Relu,
                                     bias=lb2_t[:])
                p3 = pshd.tile([1, EC], F32, tag="mm3")
                nc.tensor.matmul(p3[:, :w_], lhsT=lw3_t[:], rhs=z2[:, :w_],
                                 start=True, stop=True)
                z3 = hpool.tile([1, EC], F32, tag="z3")
                nc.vector.tensor_tensor(z3[:, :w_], p3[:, :w_],
                                        lb3_t[:].broadcast_to([1, w_]),
                                        op=mybir.AluOpType.add)
                nc.sync.dma_start(out_d[:, o:o + w_], z3[:, :w_])

    nc.compile()
    return nc


# ----------------------------------------------------------------------------
# entry point
# ----------------------------------------------------------------------------

def kernel(x, edge_index, W1, b1, W2, b2, lw1, lb1, lw2, lb2, lw3, lb3,
           _want_trace=False):
    x = np.asarray(x, np.float32)
    edge_index = np.asarray(edge_index)
    n = x.shape[0]
    npad = ((n // 2 + WIN - 1) // WIN) * WIN * 2

    dinv, plan, cores = preprocess(n, edge_index)
    shard, dpad = plan["shard"], plan["dpad"]
    nptiles = dpad // 128

    xt = np.zeros((128, npad), ml_dtypes.bfloat16)
    xt[:, :n] = (x * dinv[:, None]).T.astype(ml_dtypes.bfloat16)
    hz = np.zeros((npad // 2, 128), ml_dtypes.bfloat16)
    iota = np.tile(np.arange(DTILE, dtype=np.float32), (128, 1)).astype(
        ml_dtypes.bfloat16)

    dinvc = np.zeros((128, NCORES * nptiles), np.float32)
    for cc in range(NCORES):
        for t in range(nptiles):
            n0 = cc * shard + t * 128
            cnt = min(128, (cc + 1) * shard - n0)
            dinvc[:cnt, cc * nptiles + t] = dinv[n0:n0 + cnt]

    in_maps = []
    for c in range(NCORES):
        dinvrep = np.zeros((64, dpad), np.float32)
        dinvrep[:, :shard] = dinv[c * shard:(c + 1) * shard][None, :]
        in_maps.append({
            "xt": xt, "h1buf": hz, "h2buf": hz,
            "gidx": cores[c]["gidx"], "dstl": cores[c]["dstl"],
            "w1": np.asarray(W1, np.float32).astype(ml_dtypes.bfloat16),
            "w2": np.asarray(W2, np.float32).astype(ml_dtypes.bfloat16),
            "lw1": np.ascontiguousarray(np.asarray(lw1, np.float32)),
            "lw2": np.ascontiguousarray(np.asarray(lw2, np.float32)),
            "lw3": np.ascontiguousarray(np.asarray(lw3, np.float32)),
            "b1": np.asarray(b1, np.float32).reshape(-1, 1),
            "b2": np.asarray(b2, np.float32).reshape(-1, 1),
            "lb1": np.asarray(lb1, np.float32).reshape(-1, 1),
            "lb2": np.asarray(lb2, np.float32).reshape(-1, 1),
            "lb3": np.asarray(lb3, np.float32).reshape(-1, 1),
            "iota": iota, "dinvrep": dinvrep, "dinvc": dinvc,
        })

    meta = {"n": n, "npad": npad}
    nc = build_program(meta, plan)

    res = run_bass_kernel_spmd(nc, in_maps, core_ids=list(range(NCORES)),
                               trace=_want_trace)
    out = np.empty((n, 1), np.float32)
    for c in range(NCORES):
        out[c * shard:(c + 1) * shard, 0] = res.results[c]["out"][0, :shard]
    kernel._last_exec_ns = res.exec_time_ns
    return out


# revision 19
# speedup vs baseline: 1.3394x; 1.3033x over previous
"""GCN (2x GCNConv + MLP head) on 8 TRN2 NeuronCores via Bass/Tile.

Distribution (graph-parallel, per the node-sharding scheme):
  - nodes sharded by id across 8 cores (12500 each); weights replicated.
  - Phase A (replicated): h1l rows = (dinv*x) @ W1 for ALL nodes -> DRAM.
  - Conv edge phase (sharded by dst): for each core's in-edges,
    dma_gather 256B message rows by src id (4 SWDGE queues round-robin;
    pad slots carry idx=-1 so they emit no DMA descriptor), scalar-engine
    copy to bf16, then per-128-edge block a DVE-built bf16 one-hot S_dst
    ([128, 64] dst tiles) and a bf16 PE matmul accumulate aggT[64, 64]
    per dst tile in PSUM; epilogue dinv*aggT on DVE + bias on ACT.
  - AllGather of h1T shards (bf16) = the halo exchange.
  - Phase C (replicated): h2l rows = h1 @ W2 for ALL nodes -> DRAM.
  - Conv2 edge phase -> h2T (f32, SBUF resident).
  - MLP head in transposed space; output row [1, shard].

Host preprocessing is structure-only (derived from edge_index): degrees,
edge blocking by (dst-tile, src-window), int16 gather indices. All cores
share one program: block structure is padded to the max across cores.
Pad slots have gidx=-1 (no descriptor) and dstl=-1 (one-hot zeroes them).
"""

import numpy as np
import ml_dtypes

import concourse.bass as bass
import concourse.bacc as bacc
import concourse.tile as tile
import concourse.mybir as mybir
from concourse.bass_utils import run_bass_kernel_spmd

F32 = mybir.dt.float32
BF16 = mybir.dt.bfloat16
I16 = mybir.dt.int16

NCORES = 8
WIN = 25088          # gather window rows (multiple of 128, < int16 max)
EB = 128             # edges per block (PE contraction height)
DTILE = 64           # dst tile width (one-hot cols, PSUM agg cols)
CB = 8               # max blocks per dma_gather (1024-idx HW limit)
NQ = 4               # SWDGE queues (ucode max)
SG = 16              # chunks per coalesced index-load supergroup


# ----------------------------------------------------------------------------
# host-side preprocessing (numpy only)
# ----------------------------------------------------------------------------

def wrap16x8(a):
    """[n] int16 -> [128, n//16]: idx i at [i%16, i//16], replicated x8."""
    w = np.ascontiguousarray(np.transpose(a.reshape(-1, 16), (1, 0)))
    return np.ascontiguousarray(np.tile(w, (8, 1)))


def preprocess(n, edge_index):
    """Uniform cross-core edge plan.

    Returns (dinv, plan, cores) where plan holds the shared structure
    (chunks/blocks/flags) and cores[c] holds per-core staged index arrays.
    """
    src = edge_index[0].astype(np.int64)
    dst = edge_index[1].astype(np.int64)

    deg = np.bincount(dst, minlength=n).astype(np.float64) + 1.0
    dinv = (1.0 / np.sqrt(deg)).astype(np.float32)

    shard = n // NCORES
    assert shard * NCORES == n and shard % 2 == 0
    ntiles = (shard + DTILE - 1) // DTILE
    dpad = ntiles * DTILE
    # groups: (pair-window, src parity); pair-rows hold nodes (2r, 2r+1)
    nwin = ((n + 1) // 2 + WIN - 1) // WIN * 2

    loops = np.arange(n, dtype=np.int64)
    src = np.concatenate([src, loops])
    dst = np.concatenate([dst, loops])

    # per-core edge lists grouped by (dst tile, src window)
    per_core = []
    counts = np.zeros((NCORES, ntiles, nwin), np.int64)
    for c in range(NCORES):
        base = c * shard
        m = (dst >= base) & (dst < base + shard)
        s, d = src[m], dst[m] - base
        t_id = d // DTILE
        w_id = (s // 2) // WIN * 2 + (s % 2)
        order = np.lexsort((w_id, t_id))
        s, d, t_id, w_id = s[order], d[order], t_id[order], w_id[order]
        np.add.at(counts[c], (t_id, w_id), 1)
        per_core.append((s, d, t_id, w_id))

    nb = (counts.max(axis=0) + EB - 1) // EB      # [ntiles, nwin] blocks

    # shared chunk/block structure, tile-major
    chunks = []   # (window, n_blocks, tile)
    blocks = []   # (tile,) placeholder; start/stop/exec filled below
    for t in range(ntiles):
        for w in range(nwin):
            g = int(nb[t, w])
            b0 = 0
            while b0 < g:
                k = min(CB, g - b0)
                chunks.append((w, k, t))
                for j in range(k):
                    blocks.append(t)
                b0 += k
            done = None
    goff, boff, nidxs = [], [], []
    g0 = b0_ = 0
    cmax = counts.max(axis=0)                      # [ntiles, nwin]
    done_in_grp = {}
    for (w, k, t) in chunks:
        goff.append(g0); boff.append(b0_)
        b0 = done_in_grp.get((t, w), 0)
        real = int(min(max(cmax[t, w] - b0 * EB, 1), k * EB))
        nidxs.append((real + 15) // 16 * 16)
        done_in_grp[(t, w)] = b0 + k
        g0 += k * EB // 16
        b0_ += k

    # per-core staged arrays
    cores = []
    for c in range(NCORES):
        s, d, t_id, w_id = per_core[c]
        gidx = np.zeros((b0_ * EB,), np.int16)           # pad: window row 0
        dstl = np.full((b0_ * EB,), -1.0, np.float32)    # pad: matches no dst
        # locate each core group inside the shared layout
        key = t_id * nwin + w_id
        cuts = np.flatnonzero(np.diff(key)) + 1
        starts = np.concatenate([[0], cuts]) if len(s) else np.array([], np.int64)
        ends = np.concatenate([cuts, [len(s)]]) if len(s) else np.array([], np.int64)
        # block offset of group (t, w) in the shared layout
        grp_boff = np.zeros((ntiles, nwin), np.int64)
        acc = 0
        for t in range(ntiles):
            for w in range(nwin):
                grp_boff[t, w] = acc
                acc += nb[t, w]
        for a, b in zip(starts, ends):
            t = int(t_id[a]); w = int(w_id[a])
            o = grp_boff[t, w] * EB
            cnt = b - a
            gidx[o:o + cnt] = ((s[a:b] // 2) - (w // 2) * WIN).astype(np.int16)
            dstl[o:o + cnt] = (d[a:b] - t * DTILE).astype(np.float32)
        cores.append(dict(
            gidx=wrap16x8(gidx),
            dstl=np.ascontiguousarray(
                dstl.reshape(b0_, EB).T.astype(ml_dtypes.bfloat16)),
            base=c * shard,
        ))

    # executed = block index within chunk < ceil(nidx/128); start/stop per
    # tile over executed blocks only (skipped blocks would read stale SBUF)
    execf = []
    for ci, (w, k, t) in enumerate(chunks):
        kk = (nidxs[ci] + EB - 1) // EB
        execf.extend(j < kk for j in range(k))
    first, last = {}, {}
    for i, (t, e) in enumerate(zip(blocks, execf)):
        if e:
            first.setdefault(t, i)
            last[t] = i
    blocks = [(t, e, e and first[t] == i, e and last[t] == i)
              for i, (t, e) in enumerate(zip(blocks, execf))]

    plan = dict(chunks=chunks, blocks=blocks, goff=goff, boff=boff,
                nidxs=nidxs, ntiles=ntiles, dpad=dpad, shard=shard,
                nwin=nwin, gcols=g0, bcols=b0_)
    return dinv, plan, cores


# ----------------------------------------------------------------------------
# device program
# ----------------------------------------------------------------------------

def emit_conv_edges(nc, pool, gpool, ipool, psum, plan, hbuf, gidx_d, dstl_d,
                    iota_t, dinvrep_t, bias_t, out_cb):
    """One conv's edge aggregation. out_cb(tile_idx, e1_f32_tile)."""
    chunks = plan["chunks"]
    agg = {"t": None}
    sup = {}
    bi = 0
    for ci, (w, k, t) in enumerate(chunks):
        if ci % SG == 0:
            # coalesced index/dstl load for chunks [ci, ci+SG)
            hi = min(ci + SG, len(chunks))
            g0 = plan["goff"][ci]
            b0 = plan["boff"][ci]
            g1 = plan["goff"][hi - 1] + chunks[hi - 1][1] * EB // 16
            b1 = plan["boff"][hi - 1] + chunks[hi - 1][1]
            it = ipool.tile([128, SG * CB * EB // 16], I16, tag="gidx")
            nc.sync.dma_start(it[:, :g1 - g0], gidx_d[:, g0:g1])
            dl = ipool.tile([128, SG * CB], BF16, tag="dstl")
            nc.sync.dma_start(dl[:, :b1 - b0], dstl_d[:, b0:b1])
            sup = {"it": it, "dl": dl, "g0": g0, "b0": b0}
        go = plan["goff"][ci] - sup["g0"]
        bo = plan["boff"][ci] - sup["b0"]
        it, dl = sup["it"], sup["dl"]
        nidx = plan["nidxs"][ci]
        kk = (nidx + EB - 1) // EB
        pw, half = w >> 1, w & 1
        g = gpool.tile([128, CB, 128], BF16, tag="g")
        nc.gpsimd.dma_gather(
            g[:, :kk, :],
            hbuf[pw * WIN:(pw + 1) * WIN, :],
            it[:, go:go + nidx // 16],
            num_idxs=nidx, num_idxs_reg=nidx, elem_size=128,
            queue_num=ci % NQ,
        )
        s_t = gpool.tile([128, CB, DTILE], BF16, tag="s")
        nc.vector.tensor_tensor(
            s_t[:, :k, :],
            iota_t[:].unsqueeze(1).broadcast_to([128, k, DTILE]),
            dl[:, bo:bo + k].unsqueeze(2).broadcast_to([128, k, DTILE]),
            op=mybir.AluOpType.is_equal,
        )
        for j in range(k):
            t_, ex, start, stop = plan["blocks"][bi]; bi += 1
            if not ex:
                continue
            if start:
                agg["t"] = psum.tile([64, DTILE], F32, tag="agg",
                                     name=f"agg_{bi}")
            nc.tensor.matmul(agg["t"][:],
                             lhsT=g[:, j, half * 64:(half + 1) * 64],
                             rhs=s_t[:, j, :], start=start, stop=stop)
            if stop:
                ag = agg["t"]
                e1 = pool.tile([64, DTILE], F32, tag="ep1")
                nc.vector.tensor_tensor(
                    e1[:], ag[:],
                    dinvrep_t[:, t_ * DTILE:(t_ + 1) * DTILE],
                    op=mybir.AluOpType.mult)
                out_cb(t_, e1)


def build_program(meta, plan):
    n = meta["n"]
    npad = meta["npad"]
    dpad = plan["dpad"]
    shard = plan["shard"]
    ntiles = plan["ntiles"]
    nptiles = dpad // 128
    gcols = max(plan["gcols"], 16)
    bcols = max(plan["bcols"], 1)

    nc = bacc.Bacc("TRN2", target_bir_lowering=False, debug=False,
                   num_devices=NCORES, num_swdge_queues=NQ)

    xt = nc.dram_tensor("xt", [128, npad], BF16, kind="ExternalInput")
    h1buf = nc.dram_tensor("h1buf", [npad // 2, 128], BF16,
                           kind="ExternalInput")
    h2buf = nc.dram_tensor("h2buf", [npad // 2, 128], BF16,
                           kind="ExternalInput")
    gidx_d = nc.dram_tensor("gidx", [128, gcols], I16, kind="ExternalInput")
    dstl_d = nc.dram_tensor("dstl", [128, bcols], BF16, kind="ExternalInput")
    w1_d = nc.dram_tensor("w1", [128, 64], BF16, kind="ExternalInput")
    w2_d = nc.dram_tensor("w2", [64, 64], BF16, kind="ExternalInput")
    lw1_d = nc.dram_tensor("lw1", [64, 64], F32, kind="ExternalInput")
    lw2_d = nc.dram_tensor("lw2", [64, 32], F32, kind="ExternalInput")
    lw3_d = nc.dram_tensor("lw3", [32, 1], F32, kind="ExternalInput")
    b1_d = nc.dram_tensor("b1", [64, 1], F32, kind="ExternalInput")
    b2_d = nc.dram_tensor("b2", [64, 1], F32, kind="ExternalInput")
    lb1_d = nc.dram_tensor("lb1", [64, 1], F32, kind="ExternalInput")
    lb2_d = nc.dram_tensor("lb2", [32, 1], F32, kind="ExternalInput")
    lb3_d = nc.dram_tensor("lb3", [1, 1], F32, kind="ExternalInput")
    iota_d = nc.dram_tensor("iota", [128, DTILE], BF16, kind="ExternalInput")
    dinvrep_d = nc.dram_tensor("dinvrep", [64, dpad], F32, kind="ExternalInput")
    dinvc_d = nc.dram_tensor("dinvc", [128, NCORES * nptiles], F32,
                             kind="ExternalInput")
    out_d = nc.dram_tensor("out", [1, dpad], F32, kind="ExternalOutput")

    AC = 512  # phase A/C node-chunk

    with tile.TileContext(nc) as tc:
        with (
            tc.tile_pool(name="const", bufs=1) as cpool,
            tc.tile_pool(name="work", bufs=6) as pool,
            tc.tile_pool(name="gat", bufs=10) as gpool,
            tc.tile_pool(name="gat", bufs=10) as gpool,
            tc.tile_pool(name="head", bufs=3) as hpool,
            tc.tile_pool(name="idx", bufs=3) as ipool,
            tc.tile_pool(name="xtp", bufs=2) as xtpool,
            tc.tile_pool(name="psag", bufs=3, space="PSUM") as psag,
            tc.tile_pool(name="psmm", bufs=2, space="PSUM") as psmm,
            tc.tile_pool(name="pshd", bufs=1, space="PSUM") as pshd,
            tc.tile_pool(name="dram", bufs=1, space="DRAM") as dram,
        ):
            def load_const(dram_t, shape, dtype, tag):
                t = cpool.tile(shape, dtype, tag=tag)
                nc.sync.dma_start(t[:], dram_t[:])
                return t

            w1_t = load_const(w1_d, [128, 64], BF16, "w1")
            w2_t = load_const(w2_d, [64, 64], BF16, "w2")
            lw1_t = load_const(lw1_d, [64, 64], F32, "lw1")
            lw2_t = load_const(lw2_d, [64, 32], F32, "lw2")
            lw3_t = load_const(lw3_d, [32, 1], F32, "lw3")
            b1_t = load_const(b1_d, [64, 1], F32, "b1")
            b2_t = load_const(b2_d, [64, 1], F32, "b2")
            lb1_t = load_const(lb1_d, [64, 1], F32, "lb1")
            lb2_t = load_const(lb2_d, [32, 1], F32, "lb2")
            lb3_t = load_const(lb3_d, [1, 1], F32, "lb3")
            iota_t = load_const(iota_d, [128, DTILE], BF16, "iota")
            dinvrep_t = load_const(dinvrep_d, [64, dpad], F32, "dinvrep")
            dinvc_t = load_const(dinvc_d, [128, NCORES * nptiles], F32, "dinvc")

            out_engines = [nc.scalar, nc.gpsimd, nc.sync]

            # --- phase A ---
            for t in range(npad // AC):
                st = xtpool.tile([128, AC], BF16, tag="xt")
                out_engines[t % 3].dma_start(st[:], xt[:, t * AC:(t + 1) * AC])
                for j in range(AC // 128):
                    ps = psmm.tile([128, 64], F32, tag="mm")
                    nc.tensor.matmul(
                        ps[:], lhsT=st[:, j * 128:(j + 1) * 128],
                        rhs=w1_t[:], start=True, stop=True)
                    sb = pool.tile([128, 64], BF16, tag="arow")
                    if j % 2 == 0:
                        nc.vector.tensor_copy(sb[:], ps[:])
                    else:
                        nc.scalar.activation(sb[:], ps[:],
                                             mybir.ActivationFunctionType.Copy)
                    r = t * AC + j * 128
                    out_engines[(t + j + 1) % 3].dma_start(
                        h1buf[r // 2:r // 2 + 64, :], sb[:])

            # --- conv1 edges -> h1T bf16 bounce ---
            h1t_bounce = dram.tile([64, dpad], BF16)
            ag_out = dram.tile([NCORES * 64, dpad], BF16, addr_space="Shared")

            def conv1_out(t_, e1):
                e2 = pool.tile([64, DTILE], BF16, tag="ep2")
                nc.scalar.activation(e2[:], e1[:],
                                     mybir.ActivationFunctionType.Identity,
                                     bias=b1_t[:])
                nc.scalar.dma_start(h1t_bounce[:, t_ * DTILE:(t_ + 1) * DTILE],
                                    e2[:])

            for _mi in range(10):
                gz = gpool.tile([128, CB, 128], BF16, tag="g", name=f"gz{_mi}")
                nc.vector.memset(gz[:], 0.0)

            emit_conv_edges(nc, pool, gpool, ipool, psag, plan, h1buf, gidx_d,
                            dstl_d, iota_t, dinvrep_t, b1_t, conv1_out)

            if dpad > shard:
                zt = pool.tile([64, dpad - shard], BF16, tag="zt")
                nc.vector.memset(zt[:], 0.0)
                nc.sync.dma_start(h1t_bounce[:, shard:], zt[:])

            nc.gpsimd.collective_compute(
                "AllGather", mybir.AluOpType.bypass,
                ins=[h1t_bounce[:].opt()],
                outs=[ag_out[:].opt()],
                replica_groups=[list(range(NCORES))],
            )

            # --- phase C: h2l rows for all nodes ---
            for c in range(NCORES):
                for o in range(0, shard, AC):
                    cw = min(AC, shard - o)
                    st = xtpool.tile([64, AC], BF16, tag="ct")
                    out_engines[(o // AC) % 3].dma_start(
                        st[:, :cw], ag_out[c * 64:(c + 1) * 64, o:o + cw])
                    for j in range(0, cw, 128):
                        cnt = min(128, cw - j)
                        n0 = c * shard + o + j
                        t = (o + j) // 128
                        ps = psmm.tile([128, 64], F32, tag="mm")
                        nc.tensor.matmul(ps[:cnt, :], lhsT=st[:, j:j + cnt],
                                         rhs=w2_t[:], start=True, stop=True)
                        sb = pool.tile([128, 64], BF16, tag="crow")
                        dslice = dinvc_t[:cnt, c * nptiles + t:c * nptiles + t + 1]
                        if (j // 128) % 2 == 0:
                            nc.vector.tensor_tensor(
                                sb[:cnt, :], ps[:cnt, :],
                                dslice.broadcast_to([cnt, 64]),
                                op=mybir.AluOpType.mult)
                        else:
                            nc.scalar.activation(
                                sb[:cnt, :], ps[:cnt, :],
                                mybir.ActivationFunctionType.Copy,
                                scale=dslice)
                        out_engines[(j // 128 + 1) % 3].dma_start(
                            h2buf[n0 // 2:(n0 + cnt) // 2, :], sb[:cnt, :])

            # --- conv2 edges -> h2T f32 in SBUF (per-head-chunk tiles) ---
            EC = 512
            nhc = (dpad + EC - 1) // EC
            h2t_tiles = [
                cpool.tile([64, min(EC, dpad - o * EC)], F32, tag=f"h2t{o}",
                           name=f"h2t{o}")
                for o in range(nhc)
            ]

            def conv2_out(t_, e1):
                col = t_ * DTILE
                hc, off = col // EC, col % EC
                nc.scalar.activation(h2t_tiles[hc][:, off:off + DTILE],
                                     e1[:],
                                     mybir.ActivationFunctionType.Identity,
                                     bias=b2_t[:])

            emit_conv_edges(nc, pool, gpool, ipool, psag, plan, h2buf, gidx_d,
                            dstl_d, iota_t, dinvrep_t, b2_t, conv2_out)

            # --- MLP head (transposed space) ---
            for hc in range(nhc):
                o = hc * EC
                w_ = min(EC, dpad - o)
                p1 = pshd.tile([64, EC], F32, tag="mm1")
                nc.tensor.matmul(p1[:, :w_], lhsT=lw1_t[:],
                                 rhs=h2t_tiles[hc][:, :w_], start=True,
                                 stop=True)
                z1 = hpool.tile([64, EC], F32, tag="z1")
                nc.scalar.activation(z1[:, :w_], p1[:, :w_],
                                     mybir.ActivationFunctionType.Relu,
                                     bias=lb1_t[:])
                p2 = pshd.tile([32, EC], F32, tag="mm2")
                nc.tensor.matmul(p2[:, :w_], lhsT=lw2_t[:], rhs=z1[:, :w_],
                                 start=True, stop=True)
                z2 = hpool.tile([32, EC], F32, tag="z2")
                nc.scalar.activation(z2[:, :w_], p2[:, :w_],
                                     mybir.ActivationFunctionType.Relu,
                                     bias=lb2_t[:])
                p3 = pshd.tile([1, EC], F32, tag="mm3")
                nc.tensor.matmul(p3[:, :w_], lhsT=lw3_t[:], rhs=z2[:, :w_],
                                 start=True, stop=True)
                z3 = hpool.tile([1, EC], F32, tag="z3")
                nc.vector.tensor_tensor(z3[:, :w_], p3[:, :w_],
                                        lb3_t[:].broadcast_to([1, w_]),
                                        op=mybir.AluOpType.add)
                nc.sync.dma_start(out_d[:, o:o + w_], z3[:, :w_])

    nc.compile()
    return nc


# ----------------------------------------------------------------------------
# entry point
# ----------------------------------------------------------------------------

def kernel(x, edge_index, W1, b1, W2, b2, lw1, lb1, lw2, lb2, lw3, lb3,
           _want_trace=False):
    x = np.asarray(x, np.float32)
    edge_index = np.asarray(edge_index)
    n = x.shape[0]
    npad = ((n + WIN - 1) // WIN) * WIN

    dinv, plan, cores = preprocess(n, edge_index)
    shard, dpad, ntiles = plan["shard"], plan["dpad"], plan["ntiles"]
    nptiles = dpad // 128

    xt = np.zeros((128, npad), ml_dtypes.bfloat16)
    xt[:, :n] = (x * dinv[:, None]).T.astype(ml_dtypes.bfloat16)
    hz = np.zeros((npad // 2, 128), ml_dtypes.bfloat16)
    iota = np.tile(np.arange(DTILE, dtype=np.float32), (128, 1)).astype(
        ml_dtypes.bfloat16)

    dinvc = np.zeros((128, NCORES * nptiles), np.float32)
    for cc in range(NCORES):
        for t in range(nptiles):
            n0 = cc * shard + t * 128
            cnt = min(128, (cc + 1) * shard - n0)
            dinvc[:cnt, cc * nptiles + t] = dinv[n0:n0 + cnt]

    in_maps = []
    for c in range(NCORES):
        dinvrep = np.zeros((64, dpad), np.float32)
        dinvrep[:, :shard] = dinv[c * shard:(c + 1) * shard][None, :]
        in_maps.append({
            "xt": xt, "h1buf": hz, "h2buf": hz,
            "gidx": cores[c]["gidx"], "dstl": cores[c]["dstl"],
            "w1": np.asarray(W1, np.float32).astype(ml_dtypes.bfloat16),
            "w2": np.asarray(W2, np.float32).astype(ml_dtypes.bfloat16),
            "lw1": np.ascontiguousarray(np.asarray(lw1, np.float32)),
            "lw2": np.ascontiguousarray(np.asarray(lw2, np.float32)),
            "lw3": np.ascontiguousarray(np.asarray(lw3, np.float32)),
            "b1": np.asarray(b1, np.float32).reshape(-1, 1),
            "b2": np.asarray(b2, np.float32).reshape(-1, 1),
            "lb1": np.asarray(lb1, np.float32).reshape(-1, 1),
            "lb2": np.asarray(lb2, np.float32).reshape(-1, 1),
            "lb3": np.asarray(lb3, np.float32).reshape(-1, 1),
            "iota": iota, "dinvrep": dinvrep, "dinvc": dinvc,
        })

    meta = {"n": n, "npad": npad}
    nc = build_program(meta, plan)

    res = run_bass_kernel_spmd(nc, in_maps, core_ids=list(range(NCORES)),
                               trace=_want_trace)
    out = np.empty((n, 1), np.float32)
    for c in range(NCORES):
        out[c * shard:(c + 1) * shard, 0] = res.results[c]["out"][0, :shard]
    kernel._last_exec_ns = res.exec_time_ns
    return out


# revision 24
# speedup vs baseline: 1.7613x; 1.3150x over previous
"""GCN (2x GCNConv + MLP head) on 8 TRN2 NeuronCores via Bass/Tile.

Distribution (graph-parallel, per the node-sharding scheme):
  - nodes sharded by id across 8 cores (12500 each); weights replicated.
  - Phase A (replicated): h1l rows = (dinv*x) @ W1 for ALL nodes -> DRAM.
  - Conv edge phase (sharded by dst): for each core's in-edges,
    dma_gather 256B message rows by src id (4 SWDGE queues round-robin;
    pad slots carry idx=-1 so they emit no DMA descriptor), scalar-engine
    copy to bf16, then per-128-edge block a DVE-built bf16 one-hot S_dst
    ([128, 64] dst tiles) and a bf16 PE matmul accumulate aggT[64, 64]
    per dst tile in PSUM; epilogue dinv*aggT on DVE + bias on ACT.
  - AllGather of h1T shards (bf16) = the halo exchange.
  - Phase C (replicated): h2l rows = h1 @ W2 for ALL nodes -> DRAM.
  - Conv2 edge phase -> h2T (f32, SBUF resident).
  - MLP head in transposed space; output row [1, shard].

Host preprocessing is structure-only (derived from edge_index): degrees,
edge blocking by (dst-tile, src-window), int16 gather indices. All cores
share one program: block structure is padded to the max across cores.
Pad slots have gidx=-1 (no descriptor) and dstl=-1 (one-hot zeroes them).
"""

import numpy as np
import ml_dtypes

import concourse.bass as bass
import concourse.bacc as bacc
import concourse.tile as tile
import concourse.mybir as mybir
from concourse.bass_utils import run_bass_kernel_spmd

F32 = mybir.dt.float32
BF16 = mybir.dt.bfloat16
I16 = mybir.dt.int16

NCORES = 8
WIN = 25088          # gather window rows (multiple of 128, < int16 max)
EB = 128             # edges per block (PE contraction height)
DTILE = 64           # dst tile width (one-hot cols, PSUM agg cols)
CB = 8               # max blocks per dma_gather (1024-idx HW limit)
NQ = 4               # SWDGE queues (ucode max)
SG = 16              # chunks per coalesced index-load supergroup


# ----------------------------------------------------------------------------
# host-side preprocessing (numpy only)
# ----------------------------------------------------------------------------

def wrap16x8(a):
    """[n] int16 -> [128, n//16]: idx i at [i%16, i//16], replicated x8."""
    w = np.ascontiguousarray(np.transpose(a.reshape(-1, 16), (1, 0)))
    return np.ascontiguousarray(np.tile(w, (8, 1)))


def preprocess(n, edge_index):
    """Uniform cross-core edge plan.

    Returns (dinv, plan, cores) where plan holds the shared structure
    (chunks/blocks/flags) and cores[c] holds per-core staged index arrays.
    """
    src = edge_index[0].astype(np.int64)
    dst = edge_index[1].astype(np.int64)

    deg = np.bincount(dst, minlength=n).astype(np.float64) + 1.0
    dinv = (1.0 / np.sqrt(deg)).astype(np.float32)

    shard = n // NCORES
    assert shard * NCORES == n and shard % 2 == 0
    ntiles = (shard + DTILE - 1) // DTILE
    dpad = ntiles * DTILE
    # h layout: per-core-padded pair-rows. Node s (core cs, local l) lives
    # at pair-row cs*(dpad//2) + l//2, half l%2. Windows of WIN pair-rows
    # align to 4-core groups (4 * dpad//2 == WIN * ... ). Groups:
    # (pair-window, parity).
    hpc = dpad // 2
    nwin = (NCORES * hpc + WIN - 1) // WIN * 2

    loops = np.arange(n, dtype=np.int64)
    src = np.concatenate([src, loops])
    dst = np.concatenate([dst, loops])

    # per-core edge lists grouped by (dst tile, src window)
    per_core = []
    counts = np.zeros((NCORES, ntiles, nwin), np.int64)
    for c in range(NCORES):
        base = c * shard
        m = (dst >= base) & (dst < base + shard)
        s, d = src[m], dst[m] - base
        t_id = d // DTILE
        prow = (s // shard) * hpc + (s % shard) // 2
        w_id = (prow // WIN) * 2 + (s % 2)
        order = np.lexsort((w_id, t_id))
        s, d, t_id, w_id, prow = (s[order], d[order], t_id[order],
                                  w_id[order], prow[order])
        np.add.at(counts[c], (t_id, w_id), 1)
        per_core.append((s, d, t_id, w_id, prow))

    nb = (counts.max(axis=0) + EB - 1) // EB      # [ntiles, nwin] blocks

    # shared chunk/block structure, tile-major
    chunks = []   # (window, n_blocks, tile)
    blocks = []   # (tile,) placeholder; start/stop/exec filled below
    for t in range(ntiles):
        for w in range(nwin):
            g = int(nb[t, w])
            b0 = 0
            while b0 < g:
                k = min(CB, g - b0)
                chunks.append((w, k, t))
                for j in range(k):
                    blocks.append(t)
                b0 += k
            done = None
    goff, boff, nidxs = [], [], []
    g0 = b0_ = 0
    cmax = counts.max(axis=0)                      # [ntiles, nwin]
    done_in_grp = {}
    for (w, k, t) in chunks:
        goff.append(g0); boff.append(b0_)
        b0 = done_in_grp.get((t, w), 0)
        real = int(min(max(cmax[t, w] - b0 * EB, 1), k * EB))
        nidxs.append((real + 15) // 16 * 16)
        done_in_grp[(t, w)] = b0 + k
        g0 += k * EB // 16
        b0_ += k

    # per-core staged arrays
    cores = []
    for c in range(NCORES):
        s, d, t_id, w_id, prow = per_core[c]
        gidx = np.zeros((b0_ * EB,), np.int16)           # pad: window row 0
        dstl = np.full((b0_ * EB,), -1.0, np.float32)    # pad: matches no dst
        # locate each core group inside the shared layout
        key = t_id * nwin + w_id
        cuts = np.flatnonzero(np.diff(key)) + 1
        starts = np.concatenate([[0], cuts]) if len(s) else np.array([], np.int64)
        ends = np.concatenate([cuts, [len(s)]]) if len(s) else np.array([], np.int64)
        # block offset of group (t, w) in the shared layout
        grp_boff = np.zeros((ntiles, nwin), np.int64)
        acc = 0
        for t in range(ntiles):
            for w in range(nwin):
                grp_boff[t, w] = acc
                acc += nb[t, w]
        for a, b in zip(starts, ends):
            t = int(t_id[a]); w = int(w_id[a])
            o = grp_boff[t, w] * EB
            cnt = b - a
            gidx[o:o + cnt] = (prow[a:b] - (w // 2) * WIN).astype(np.int16)
            dstl[o:o + cnt] = (d[a:b] - t * DTILE).astype(np.float32)
        cores.append(dict(
            gidx=wrap16x8(gidx),
            dstl=np.ascontiguousarray(
                dstl.reshape(b0_, EB).T.astype(ml_dtypes.bfloat16)),
            base=c * shard,
        ))

    # executed = block index within chunk < ceil(nidx/128); start/stop per
    # tile over executed blocks only (skipped blocks would read stale SBUF)
    execf = []
    for ci, (w, k, t) in enumerate(chunks):
        kk = (nidxs[ci] + EB - 1) // EB
        execf.extend(j < kk for j in range(k))
    first, last = {}, {}
    for i, (t, e) in enumerate(zip(blocks, execf)):
        if e:
            first.setdefault(t, i)
            last[t] = i
    blocks = [(t, e, e and first[t] == i, e and last[t] == i)
              for i, (t, e) in enumerate(zip(blocks, execf))]

    plan = dict(chunks=chunks, blocks=blocks, goff=goff, boff=boff,
                nidxs=nidxs, ntiles=ntiles, dpad=dpad, shard=shard,
                nwin=nwin, gcols=g0, bcols=b0_)
    return dinv, plan, cores


# ----------------------------------------------------------------------------
# device program
# ----------------------------------------------------------------------------

def emit_conv_edges(nc, pool, gpool, ipool, psum, plan, hbuf, gidx_d, dstl_d,
                    iota_t, dinvrep_t, bias_t, out_cb):
    """One conv's edge aggregation. out_cb(tile_idx, e1_f32_tile)."""
    chunks = plan["chunks"]
    agg = {"t": None}
    sup = {}
    bi = 0
    for ci, (w, k, t) in enumerate(chunks):
        if ci % SG == 0:
            # coalesced index/dstl load for chunks [ci, ci+SG)
            hi = min(ci + SG, len(chunks))
            g0 = plan["goff"][ci]
            b0 = plan["boff"][ci]
            g1 = plan["goff"][hi - 1] + chunks[hi - 1][1] * EB // 16
            b1 = plan["boff"][hi - 1] + chunks[hi - 1][1]
            it = ipool.tile([128, SG * CB * EB // 16], I16, tag="gidx")
            nc.sync.dma_start(it[:, :g1 - g0], gidx_d[:, g0:g1])
            dl = ipool.tile([128, SG * CB], BF16, tag="dstl")
            nc.sync.dma_start(dl[:, :b1 - b0], dstl_d[:, b0:b1])
            sup = {"it": it, "dl": dl, "g0": g0, "b0": b0}
        go = plan["goff"][ci] - sup["g0"]
        bo = plan["boff"][ci] - sup["b0"]
        it, dl = sup["it"], sup["dl"]
        nidx = plan["nidxs"][ci]
        kk = (nidx + EB - 1) // EB
        pw, half = w >> 1, w & 1
        g = gpool.tile([128, CB, 128], BF16, tag="g")
        nc.gpsimd.dma_gather(
            g[:, :kk, :],
            hbuf[pw * WIN:(pw + 1) * WIN, :],
            it[:, go:go + nidx // 16],
            num_idxs=nidx, num_idxs_reg=nidx, elem_size=128,
            queue_num=ci % NQ,
        )
        s_t = gpool.tile([128, CB, DTILE], BF16, tag="s")
        nc.vector.tensor_tensor(
            s_t[:, :k, :],
            iota_t[:].unsqueeze(1).broadcast_to([128, k, DTILE]),
            dl[:, bo:bo + k].unsqueeze(2).broadcast_to([128, k, DTILE]),
            op=mybir.AluOpType.is_equal,
        )
        for j in range(k):
            t_, ex, start, stop = plan["blocks"][bi]; bi += 1
            if not ex:
                continue
            if start:
                agg["t"] = psum.tile([64, DTILE], F32, tag="agg",
                                     name=f"agg_{bi}")
            nc.tensor.matmul(agg["t"][:],
                             lhsT=g[:, j, half * 64:(half + 1) * 64],
                             rhs=s_t[:, j, :], start=start, stop=stop)
            if stop:
                ag = agg["t"]
                e1 = pool.tile([64, DTILE], F32, tag="ep1")
                nc.vector.tensor_tensor(
                    e1[:], ag[:],
                    dinvrep_t[:, t_ * DTILE:(t_ + 1) * DTILE],
                    op=mybir.AluOpType.mult)
                out_cb(t_, e1)


def build_program(meta, plan):
    n = meta["n"]
    npad = meta["npad"]
    dpad = plan["dpad"]
    shard = plan["shard"]
    ntiles = plan["ntiles"]
    nptiles = dpad // 128
    gcols = max(plan["gcols"], 16)
    bcols = max(plan["bcols"], 1)

    nc = bacc.Bacc("TRN2", target_bir_lowering=False, debug=False,
                   num_devices=NCORES, num_swdge_queues=NQ)

    hpc = dpad // 2
    xt = nc.dram_tensor("xt", [128, shard], BF16, kind="ExternalInput")
    gidx_d = nc.dram_tensor("gidx", [128, gcols], I16, kind="ExternalInput")
    dstl_d = nc.dram_tensor("dstl", [128, bcols], BF16, kind="ExternalInput")
    w1_d = nc.dram_tensor("w1", [128, 64], BF16, kind="ExternalInput")
    w2_d = nc.dram_tensor("w2", [64, 64], BF16, kind="ExternalInput")
    lw1_d = nc.dram_tensor("lw1", [64, 64], F32, kind="ExternalInput")
    lw2_d = nc.dram_tensor("lw2", [64, 32], F32, kind="ExternalInput")
    lw3_d = nc.dram_tensor("lw3", [32, 1], F32, kind="ExternalInput")
    b1_d = nc.dram_tensor("b1", [64, 1], F32, kind="ExternalInput")
    b2_d = nc.dram_tensor("b2", [64, 1], F32, kind="ExternalInput")
    lb1_d = nc.dram_tensor("lb1", [64, 1], F32, kind="ExternalInput")
    lb2_d = nc.dram_tensor("lb2", [32, 1], F32, kind="ExternalInput")
    lb3_d = nc.dram_tensor("lb3", [1, 1], F32, kind="ExternalInput")
    iota_d = nc.dram_tensor("iota", [128, DTILE], BF16, kind="ExternalInput")
    dinvrep_d = nc.dram_tensor("dinvrep", [64, dpad], F32, kind="ExternalInput")
    dinvc_d = nc.dram_tensor("dinvc", [128, nptiles], F32,
                             kind="ExternalInput")
    out_d = nc.dram_tensor("out", [1, dpad], F32, kind="ExternalOutput")

    AC = 512  # phase A/C node-chunk

    with tile.TileContext(nc) as tc:
        with (
            tc.tile_pool(name="const", bufs=1) as cpool,
            tc.tile_pool(name="work", bufs=6) as pool,
            tc.tile_pool(name="gat", bufs=10) as gpool,
            tc.tile_pool(name="gat", bufs=10) as gpool,
            tc.tile_pool(name="head", bufs=3) as hpool,
            tc.tile_pool(name="idx", bufs=3) as ipool,
            tc.tile_pool(name="xtp", bufs=2) as xtpool,
            tc.tile_pool(name="psag", bufs=3, space="PSUM") as psag,
            tc.tile_pool(name="psmm", bufs=2, space="PSUM") as psmm,
            tc.tile_pool(name="pshd", bufs=1, space="PSUM") as pshd,
            tc.tile_pool(name="dram", bufs=1, space="DRAM") as dram,
        ):
            def load_const(dram_t, shape, dtype, tag):
                t = cpool.tile(shape, dtype, tag=tag)
                nc.sync.dma_start(t[:], dram_t[:])
                return t

            w1_t = load_const(w1_d, [128, 64], BF16, "w1")
            w2_t = load_const(w2_d, [64, 64], BF16, "w2")
            lw1_t = load_const(lw1_d, [64, 64], F32, "lw1")
            lw2_t = load_const(lw2_d, [64, 32], F32, "lw2")
            lw3_t = load_const(lw3_d, [32, 1], F32, "lw3")
            b1_t = load_const(b1_d, [64, 1], F32, "b1")
            b2_t = load_const(b2_d, [64, 1], F32, "b2")
            lb1_t = load_const(lb1_d, [64, 1], F32, "lb1")
            lb2_t = load_const(lb2_d, [32, 1], F32, "lb2")
            lb3_t = load_const(lb3_d, [1, 1], F32, "lb3")
            iota_t = load_const(iota_d, [128, DTILE], BF16, "iota")
            dinvrep_t = load_const(dinvrep_d, [64, dpad], F32, "dinvrep")
            dinvc_t = load_const(dinvc_d, [128, nptiles], F32, "dinvc")

            out_engines = [nc.scalar, nc.gpsimd, nc.sync]

            # local halo-exchange sources (collectives cannot read IO
            # tensors); pad pair-rows [shard/2, hpc) zeroed once
            h1loc = dram.tile([hpc, 128], BF16)
            h2loc = dram.tile([hpc, 128], BF16)
            if hpc > shard // 2:
                zpad = pool.tile([hpc - shard // 2, 128], BF16, tag="zpad")
                nc.vector.memset(zpad[:], 0.0)
                nc.sync.dma_start(h1loc[shard // 2:, :], zpad[:])
                nc.scalar.dma_start(h2loc[shard // 2:, :], zpad[:])

            # --- phase A (own shard only) ---
            for o in range(0, shard, AC):
                cw = min(AC, shard - o)
                st = xtpool.tile([128, AC], BF16, tag="xt")
                out_engines[(o // AC) % 3].dma_start(st[:, :cw],
                                                     xt[:, o:o + cw])
                for j in range(0, cw, 128):
                    cnt = min(128, cw - j)
                    ps = psmm.tile([128, 64], F32, tag="mm")
                    nc.tensor.matmul(
                        ps[:cnt, :], lhsT=st[:, j:j + cnt],
                        rhs=w1_t[:], start=True, stop=True)
                    sb = pool.tile([128, 64], BF16, tag="arow")
                    if (j // 128) % 2 == 0:
                        nc.vector.tensor_copy(sb[:cnt, :], ps[:cnt, :])
                    else:
                        nc.scalar.activation(sb[:cnt, :], ps[:cnt, :],
                                             mybir.ActivationFunctionType.Copy)
                    r = o + j
                    out_engines[(j // 128 + 1) % 3].dma_start(
                        h1loc[r // 2:(r + cnt) // 2, :], sb[:cnt, :])

            # --- halo exchange 1: all cores' h1l pair-rows ---
            h1buf = dram.tile([NCORES * hpc, 128], BF16, addr_space="Shared")
            nc.gpsimd.collective_compute(
                "AllGather", mybir.AluOpType.bypass,
                ins=[h1loc[:].opt()],
                outs=[h1buf[:].opt()],
                replica_groups=[list(range(NCORES))],
            )

            # --- conv1 edges -> h1T bf16 strips in SBUF ---
            EC = 512
            nhc = (dpad + EC - 1) // EC
            h1t_tiles = [
                cpool.tile([64, min(EC, dpad - o * EC)], BF16, tag=f"h1t{o}",
                           name=f"h1t{o}")
                for o in range(nhc)
            ]

            def conv1_out(t_, e1):
                col = t_ * DTILE
                hc, off = col // EC, col % EC
                nc.scalar.activation(h1t_tiles[hc][:, off:off + DTILE],
                                     e1[:],
                                     mybir.ActivationFunctionType.Identity,
                                     bias=b1_t[:])

            for _mi in range(10):
                gz = gpool.tile([128, CB, 128], BF16, tag="g", name=f"gz{_mi}")
                nc.vector.memset(gz[:], 0.0)

            emit_conv_edges(nc, pool, gpool, ipool, psag, plan, h1buf, gidx_d,
                            dstl_d, iota_t, dinvrep_t, b1_t, conv1_out)

            # --- phase C (own shard, from SBUF h1T strips) ---
            for o in range(0, shard, AC):
                cw = min(AC, shard - o)
                hc = o // EC
                for j in range(0, cw, 128):
                    cnt = min(128, cw - j)
                    t = (o + j) // 128
                    ps = psmm.tile([128, 64], F32, tag="mm")
                    nc.tensor.matmul(
                        ps[:cnt, :], lhsT=h1t_tiles[hc][:, j:j + cnt],
                        rhs=w2_t[:], start=True, stop=True)
                    sb = pool.tile([128, 64], BF16, tag="crow")
                    dslice = dinvc_t[:cnt, t:t + 1]
                    if (j // 128) % 2 == 0:
                        nc.vector.tensor_tensor(
                            sb[:cnt, :], ps[:cnt, :],
                            dslice.broadcast_to([cnt, 64]),
                            op=mybir.AluOpType.mult)
                    else:
                        nc.scalar.activation(
                            sb[:cnt, :], ps[:cnt, :],
                            mybir.ActivationFunctionType.Copy,
                            scale=dslice)
                    r = o + j
                    out_engines[(j // 128 + 1) % 3].dma_start(
                        h2loc[r // 2:(r + cnt) // 2, :], sb[:cnt, :])

            # --- halo exchange 2: all cores' h2l pair-rows ---
            h2buf = dram.tile([NCORES * hpc, 128], BF16, addr_space="Shared")
            nc.gpsimd.collective_compute(
                "AllGather", mybir.AluOpType.bypass,
                ins=[h2loc[:].opt()],
                outs=[h2buf[:].opt()],
                replica_groups=[list(range(NCORES))],
            )

            # --- conv2 edges -> h2T f32 in SBUF (per-head-chunk tiles) ---
            h2t_tiles = [
                cpool.tile([64, min(EC, dpad - o * EC)], F32, tag=f"h2t{o}",
                           name=f"h2t{o}")
                for o in range(nhc)
            ]

            def conv2_out(t_, e1):
                col = t_ * DTILE
                hc, off = col // EC, col % EC
                nc.scalar.activation(h2t_tiles[hc][:, off:off + DTILE],
                                     e1[:],
                                     mybir.ActivationFunctionType.Identity,
                                     bias=b2_t[:])

            emit_conv_edges(nc, pool, gpool, ipool, psag, plan, h2buf, gidx_d,
                            dstl_d, iota_t, dinvrep_t, b2_t, conv2_out)

            # --- MLP head (transposed space) ---
            for hc in range(nhc):
                o = hc * EC
                w_ = min(EC, dpad - o)
                p1 = pshd.tile([64, EC], F32, tag="mm1")
                nc.tensor.matmul(p1[:, :w_], lhsT=lw1_t[:],
                                 rhs=h2t_tiles[hc][:, :w_], start=True,
                                 stop=True)
                z1 = hpool.tile([64, EC], F32, tag="z1")
                nc.scalar.activation(z1[:, :w_], p1[:, :w_],
                                     mybir.ActivationFunctionType.Relu,
                                     bias=lb1_t[:])
                p2 = pshd.tile([32, EC], F32, tag="mm2")
                nc.tensor.matmul(p2[:, :w_], lhsT=lw2_t[:], rhs=z1[:, :w_],
                                 start=True, stop=True)
                z2 = hpool.tile([32, EC], F32, tag="z2")
                nc.scalar.activation(z2[:, :w_], p2[:, :w_],
                                     mybir.ActivationFunctionType.Relu,
                                     bias=lb2_t[:])
                p3 = pshd.tile([1, EC], F32, tag="mm3")
                nc.tensor.matmul(p3[:, :w_], lhsT=lw3_t[:], rhs=z2[:, :w_],
                                 start=True, stop=True)
                z3 = hpool.tile([1, EC], F32, tag="z3")
                nc.vector.tensor_tensor(z3[:, :w_], p3[:, :w_],
                                        lb3_t[:].broadcast_to([1, w_]),
                                        op=mybir.AluOpType.add)
                nc.sync.dma_start(out_d[:, o:o + w_], z3[:, :w_])

    nc.compile()
    return nc


# ----------------------------------------------------------------------------
# entry point
# ----------------------------------------------------------------------------

def kernel(x, edge_index, W1, b1, W2, b2, lw1, lb1, lw2, lb2, lw3, lb3,
           _want_trace=False):
    x = np.asarray(x, np.float32)
    edge_index = np.asarray(edge_index)
    n = x.shape[0]
    npad = n

    dinv, plan, cores = preprocess(n, edge_index)
    shard, dpad, ntiles = plan["shard"], plan["dpad"], plan["ntiles"]
    nptiles = dpad // 128

    xtf = (x * dinv[:, None]).T.astype(ml_dtypes.bfloat16)
    iota = np.tile(np.arange(DTILE, dtype=np.float32), (128, 1)).astype(
        ml_dtypes.bfloat16)

    in_maps = []
    for c in range(NCORES):
        dinvrep = np.zeros((64, dpad), np.float32)
        dinvrep[:, :shard] = dinv[c * shard:(c + 1) * shard][None, :]
        dinvc = np.zeros((128, nptiles), np.float32)
        for t in range(nptiles):
            n0 = c * shard + t * 128
            cnt = min(128, (c + 1) * shard - n0)
            dinvc[:cnt, t] = dinv[n0:n0 + cnt]
        xt = np.ascontiguousarray(xtf[:, c * shard:(c + 1) * shard])
        in_maps.append({
            "xt": xt,
            "gidx": cores[c]["gidx"], "dstl": cores[c]["dstl"],
            "w1": np.asarray(W1, np.float32).astype(ml_dtypes.bfloat16),
            "w2": np.asarray(W2, np.float32).astype(ml_dtypes.bfloat16),
            "lw1": np.ascontiguousarray(np.asarray(lw1, np.float32)),
            "lw2": np.ascontiguousarray(np.asarray(lw2, np.float32)),
            "lw3": np.ascontiguousarray(np.asarray(lw3, np.float32)),
            "b1": np.asarray(b1, np.float32).reshape(-1, 1),
            "b2": np.asarray(b2, np.float32).reshape(-1, 1),
            "lb1": np.asarray(lb1, np.float32).reshape(-1, 1),
            "lb2": np.asarray(lb2, np.float32).reshape(-1, 1),
            "lb3": np.asarray(lb3, np.float32).reshape(-1, 1),
            "iota": iota, "dinvrep": dinvrep, "dinvc": dinvc,
        })

    meta = {"n": n, "npad": npad}
    nc = build_program(meta, plan)

    res = run_bass_kernel_spmd(nc, in_maps, core_ids=list(range(NCORES)),
                               trace=_want_trace)
    out = np.empty((n, 1), np.float32)
    for c in range(NCORES):
        out[c * shard:(c + 1) * shard, 0] = res.results[c]["out"][0, :shard]
    kernel._last_exec_ns = res.exec_time_ns
    return out
